# revision 30
# baseline (speedup 1.0000x reference)
"""Trainium2 Bass kernel for BEiT-style attention with relative position bias.

Shapes (hardcoded): x (64, 197, 768), 12 heads x 64 dim, rpb table (732, 12).

Sharding: data-parallel over batch -- 8 batches per NeuronCore, weights
replicated. Each core processes its 8 batches in 4 pairs (moving dim 394).

Per-core dataflow (all layouts chosen so no on-device transposes are needed):
  qk^T   = W_qk @ x^T        float32r matmuls, heads pair-packed on partitions
  v_nat  = x @ W_v^T         token-major V with a fused ones-column per head
  s^T    = k_h^T.T @ q_h^T   fp16, keys on partitions
  e      = exp(s^T - 5) * exp(rpb^T)    (softmax max-subtract replaced by a
                                         constant shift; exactly cancels)
  pv     = [v_h | ones].T @ e   -> rows 0:64 unnormalized out^T, row 64 colsum
  out^T  = pv[0:64] * bcast(1/colsum)
  final  = out^T.T @ W_p^T + b  float32r, token-major output
"""

import sys

if "/opt/trn_rl_repo" not in sys.path:
    sys.path.insert(0, "/opt/trn_rl_repo")

import numpy as np

import concourse.bass as bass
import concourse.mybir as mybir
import concourse.tile as tile
from concourse import bacc
from concourse.bass_utils import run_bass_kernel_spmd

F32 = mybir.dt.float32
F16 = mybir.dt.float16
F32R = mybir.dt.float32r
AF = mybir.ActivationFunctionType

B, N, C, H, HD = 64, 197, 768, 12, 64
NCORES = 8
BC = B // NCORES          # batches per core
PAIRS = BC // 2           # batch pairs per core
TP = 2 * N                # tokens per pair (394)
T = BC * N                # tokens per core (1576)
KT = C // 128             # contraction tiles (6)
SCALE = HD ** -0.5
VW = H * (HD + 1)         # v buffer width incl. ones columns (780)
EXP_SHIFT = -5.0


def _r(x):
    return x.bitcast(F32R)


def _ktile_layout(w):
    """(768, M) -> (128, 6*M) with k-tile-major columns."""
    m = w.shape[1]
    return np.ascontiguousarray(
        w.reshape(KT, 128, m).transpose(1, 0, 2).reshape(128, KT * m)
    )


def _build_program():
    nc = bacc.Bacc("TRN2", target_bir_lowering=False, debug=False,
                   num_devices=NCORES)

    xt_d = nc.declare_dram_parameter("xt", [128, PAIRS * KT * TP], F32R, isOutput=False)
    wqk_d = nc.declare_dram_parameter("wqk", [128, KT * 12 * 128], F32R, isOutput=False)
    wv_d = nc.declare_dram_parameter("wv", [128, KT * VW], F32R, isOutput=False)
    wp_d = nc.declare_dram_parameter("wp", [128, KT * C], F32R, isOutput=False)
    rpb0_d = nc.declare_dram_parameter("rpb0", [128, H * N], F16, isOutput=False)
    rpb1_d = nc.declare_dram_parameter("rpb1", [69, H * N], F16, isOutput=False)
    qkb_d = nc.declare_dram_parameter("qkb", [128, 12], F32, isOutput=False)
    vbr_d = nc.declare_dram_parameter("vbr", [1, VW], F32, isOutput=False)
    pbr_d = nc.declare_dram_parameter("pbr", [1, C], F32, isOutput=False)
    out_d = nc.declare_dram_parameter("out", [T, C], F32, isOutput=True)

    from contextlib import ExitStack

    with tile.TileContext(nc) as tc, ExitStack() as ctx:
        consts = ctx.enter_context(tc.tile_pool(name="consts", bufs=1))
        xt_pool = ctx.enter_context(tc.tile_pool(name="xt", bufs=2))
        qk_pool = ctx.enter_context(tc.tile_pool(name="qk", bufs=1))
        v_pool = ctx.enter_context(tc.tile_pool(name="v", bufs=1))
        es_pool = ctx.enter_context(tc.tile_pool(name="es", bufs=2))
        ot_pool = ctx.enter_context(tc.tile_pool(name="ot", bufs=2))
        fs_pool = ctx.enter_context(tc.tile_pool(name="fs", bufs=2))
        rr_pool = ctx.enter_context(tc.tile_pool(name="rr", bufs=2))
        pvs_pool = ctx.enter_context(tc.tile_pool(name="pvs", bufs=6))
        rb_pool = ctx.enter_context(tc.tile_pool(name="rb", bufs=4))
        dram_pool = ctx.enter_context(tc.tile_pool(name="dsc", bufs=4, space="DRAM"))
        ps_mm = ctx.enter_context(tc.tile_pool(name="ps_mm", bufs=3, space="PSUM"))
        ps_sc = ctx.enter_context(tc.tile_pool(name="ps_sc", bufs=3, space="PSUM"))
        ps_pv = ctx.enter_context(tc.tile_pool(name="ps_pv", bufs=2, space="PSUM"))

        if True:
            wqk_t = [consts.tile([128, 12 * 128], F32R, name=f"wqk{k}")
                     for k in range(KT)]
            wv_t = [consts.tile([128, VW], F32R, name=f"wv{k}")
                    for k in range(KT)]
            wp_t = [consts.tile([128, C], F32R, name=f"wp{k}")
                    for k in range(KT)]
            # load order matters: the DMA queue is FIFO, so put the first
            # pair's dependencies (wqk0, then xt in the pair loop) ahead of
            # the bulk weights
            nc.sync.dma_start(wqk_t[0][:], wqk_d[:, 0:1536])
            qkb = consts.tile([128, 12], F32)
            nc.sync.dma_start(qkb[:], qkb_d[:])
            xt0_t = [xt_pool.tile([128, TP], F32R, tag=f"xt{k}",
                                  name=f"xt0{k}") for k in range(KT)]
            for k in range(KT):
                nc.sync.dma_start(xt0_t[k][:], xt_d[:, k * TP:(k + 1) * TP])
            for k in range(1, KT):
                nc.sync.dma_start(wqk_t[k][:],
                                  wqk_d[:, k * 1536:(k + 1) * 1536])
            for k in range(KT):
                nc.sync.dma_start(wv_t[k][:], wv_d[:, k * VW:(k + 1) * VW])
            rpb0 = consts.tile([128, H * N], F16)
            nc.sync.dma_start(rpb0[:], rpb0_d[:])
            rpb1 = consts.tile([69, H * N], F16)
            nc.sync.dma_start(rpb1[:], rpb1_d[:])
            for k in range(KT):
                nc.sync.dma_start(wp_t[k][:], wp_d[:, k * C:(k + 1) * C])
            vbr = consts.tile([128, VW], F32)
            _vb = vbr_d[:]
            nc.sync.dma_start(
                vbr[:],
                bass.AP(tensor=_vb.tensor, offset=_vb.offset,
                        ap=[[0, 128]] + list(_vb.ap[1:])),
            )
            pbr = consts.tile([128, C], F32)
            _pb = pbr_d[:]
            nc.sync.dma_start(
                pbr[:],
                bass.AP(tensor=_pb.tensor, offset=_pb.offset,
                        ap=[[0, 128]] + list(_pb.ap[1:])),
            )
            nb = consts.tile([128, 1], F32)
            nc.vector.memset(nb[:], EXP_SHIFT)

            def _make_proj(p, ot_tiles, half_idx=None):
                tok_tiles = {0: [(0, 128), (128, 69)],
                             1: [(197, 128), (325, 69)]}
                tiles = (tok_tiles[0] + tok_tiles[1]
                         if half_idx is None else tok_tiles[half_idx])

                def emit():
                    for toff, rows in tiles:
                        fs = fs_pool.tile([128, C], F32, tag="fs", name="fs")
                        for half in range(2):
                            pf = ps_mm.tile([128, TP], F32, tag="mm",
                                            name="pf")
                            for k in range(KT):
                                nc.tensor.matmul(
                                    pf[0:rows, 0:384],
                                    ot_tiles[k][:, toff:toff + rows],
                                    wp_t[k][:, half * 384:
                                             (half + 1) * 384],
                                    start=(k == 0), stop=(k == KT - 1),
                                )
                            nc.vector.tensor_add(
                                fs[0:rows, half * 384:(half + 1) * 384],
                                pf[0:rows, 0:384],
                                pbr[0:rows, half * 384:(half + 1) * 384],
                            )
                        nc.sync.dma_start(
                            out_d[p * TP + toff: p * TP + toff + rows, :],
                            fs[0:rows, :],
                        )
                return emit

            pending_proj = None
            for p in range(PAIRS):
                # per-pair x^T load (pair 0 was prefetched before the
                # bulk weight DMAs)
                if p == 0:
                    xt_t = xt0_t
                else:
                    xt_t = [xt_pool.tile([128, TP], F32R, tag=f"xt{k}",
                                         name=f"xt{k}") for k in range(KT)]
                    for k in range(KT):
                        nc.sync.dma_start(
                            xt_t[k][:],
                            xt_d[:, p * KT * TP + k * TP:
                                 p * KT * TP + (k + 1) * TP])

                # ---- qk^T: 12 M-tiles (6 q-pair tiles, then 6 k-pair tiles)
                qk_tiles = []
                for j in range(12):
                    pq = ps_mm.tile([128, TP], F32, tag="mm")
                    for k in range(KT):
                        nc.tensor.matmul(
                            pq[:],
                            wqk_t[k][:, j * 128:(j + 1) * 128],
                            xt_t[k][:],
                            start=(k == 0), stop=(k == KT - 1),
                        )
                    qj = qk_pool.tile([128, TP], F16, tag=f"qk{j}")
                    nc.vector.tensor_scalar_add(qj[:], pq[:], qkb[:, j:j + 1])
                    qk_tiles.append(qj)

                # ---- v natural (+ones cols): 4 token tiles per pair
                v_tiles = {}
                for b2 in range(2):
                    for t, (toff, rows) in enumerate(
                            [(b2 * N, 128), (b2 * N + 128, N - 128)]):
                        vt = v_pool.tile([128, VW], F16, tag=f"v{b2}{t}")
                        for half in range(2):
                            hw = VW // 2
                            pv = ps_mm.tile([128, TP], F32, tag="mm")
                            for k in range(KT):
                                nc.tensor.matmul(
                                    pv[0:rows, 0:hw],
                                    xt_t[k][:, toff:toff + rows],
                                    wv_t[k][:, half * hw:(half + 1) * hw],
                                    start=(k == 0), stop=(k == KT - 1),
                                )
                            nc.vector.tensor_add(
                                vt[0:rows, half * hw:(half + 1) * hw],
                                pv[0:rows, 0:hw],
                                vbr[0:rows, half * hw:(half + 1) * hw],
                            )
                        v_tiles[(b2, t)] = vt

                # ---- attention
                ot_tiles = [ot_pool.tile([128, TP], F32R, tag=f"ot{k}",
                                         name=f"ot{k}")
                            for k in range(KT)]
                for b2 in range(2):
                    boff = b2 * N
                    es_tiles = []
                    for h in range(H):
                        jt, hb = h // 2, (h % 2) * 64
                        psc = ps_sc.tile([128, TP], F32, tag="sc")
                        kt_tile = qk_tiles[6 + jt]
                        q_rhs = qk_tiles[jt][hb:hb + 64, boff:boff + N]
                        nc.tensor.matmul(
                            psc[:, 0:N],
                            kt_tile[hb:hb + 64, boff:boff + 128],
                            q_rhs, start=True, stop=True,
                        )
                        nc.tensor.matmul(
                            psc[0:69, N:2 * N],
                            kt_tile[hb:hb + 64, boff + 128:boff + N],
                            q_rhs, start=True, stop=True,
                        )
                        es = es_pool.tile([128, TP], F16, tag=f"es{h}")
                        nc.scalar.activation(es[:, 0:N], psc[:, 0:N],
                                             AF.Exp, bias=nb[:])
                        nc.scalar.activation(es[0:69, N:2 * N],
                                             psc[0:69, N:2 * N],
                                             AF.Exp, bias=nb[0:69])
                        nc.vector.tensor_mul(
                            es[:, 0:N], es[:, 0:N],
                            rpb0[:, h * N:(h + 1) * N])
                        nc.vector.tensor_mul(
                            es[0:69, N:2 * N], es[0:69, N:2 * N],
                            rpb1[0:69, h * N:(h + 1) * N])
                        es_tiles.append(es)

                    # pass B: 2 heads per PSUM bank; evacuate PSUM -> SBUF
                    # immediately (ACT), normalize later off-critical-path
                    dsc = dram_pool.tile([1, H * N], F32, tag="dsc")
                    pvs_tiles = []
                    for h in range(H):
                        es = es_tiles[h]
                        if h % 2 == 0:
                            ppv = ps_pv.tile([65, TP], F32, tag="pv")
                        coff = (h % 2) * N
                        nc.tensor.matmul(
                            ppv[:, coff:coff + N],
                            v_tiles[(b2, 0)][0:128, h * 65:(h + 1) * 65],
                            es[:, 0:N], start=True, stop=False,
                        )
                        nc.tensor.matmul(
                            ppv[:, coff:coff + N],
                            v_tiles[(b2, 1)][0:69, h * 65:(h + 1) * 65],
                            es[0:69, N:2 * N], start=False, stop=True,
                        )
                        if h % 2 == 1:
                            pvs = pvs_pool.tile([65, TP], F32, tag="pvs")
                            nc.scalar.copy(pvs[:], ppv[:])
                            pvs_tiles.append(pvs)
                            nc.gpsimd.dma_start(
                                dsc[0:1, (h - 1) * N:(h + 1) * N],
                                pvs[64:65, :])
                    # batched reciprocal: DRAM hop to put 12 rows on 12
                    # partitions, then one DVE reciprocal
                    rsb = rr_pool.tile([H, N], F32, tag="rsb")
                    _d = dsc[:]
                    nc.gpsimd.dma_start(
                        rsb[:],
                        bass.AP(tensor=_d.tensor, offset=_d.offset,
                                ap=[[N, H], [1, N]]))
                    rsr = rr_pool.tile([H, N], F32, tag="rsr")
                    nc.vector.reciprocal(rsr[:], rsb[:])
                    dsc2 = dram_pool.tile([H, N], F32, tag="dsc2")
                    nc.gpsimd.dma_start(dsc2[:], rsr[:])
                    for hh in range(H):
                        jt, hb = hh // 2, (hh % 2) * 64
                        rb = rb_pool.tile([64, N], F32, tag="rb")
                        _d2 = dsc2[:]
                        nc.gpsimd.dma_start(
                            rb[:],
                            bass.AP(tensor=_d2.tensor,
                                    offset=_d2.offset + hh * N,
                                    ap=[[0, 64], [1, N]]),
                        )
                        nc.vector.tensor_mul(
                            ot_tiles[jt][hb:hb + 64, boff:boff + N],
                            pvs_tiles[hh // 2][0:64, (hh % 2) * N:
                                               (hh % 2) * N + N],
                            rb[:],
                        )
                    if p == PAIRS - 1 and b2 == 1:
                        # both halves at the very end: proj-A runs on the PE
                        # while b2=1's normalize chain drains, then proj-B
                        _make_proj(p, ot_tiles, half_idx=0)()
                        _make_proj(p, ot_tiles, half_idx=1)()

                if pending_proj is not None:
                    pending_proj()
                    pending_proj = None
                if p < PAIRS - 1:
                    pending_proj = _make_proj(p, ot_tiles)

    nc.compile()
    return nc


_PROGRAM_CACHE = {}


def _get_program():
    if "nc" not in _PROGRAM_CACHE:
        _PROGRAM_CACHE["nc"] = _build_program()
    return _PROGRAM_CACHE["nc"]


def _host_prep(x, qkv_w, q_bias, v_bias, rpb_table, proj_w, proj_b,
               rel_pos_index):
    x = np.asarray(x, dtype=np.float32)
    qkv_w = np.asarray(qkv_w, dtype=np.float32)
    q_bias = np.asarray(q_bias, dtype=np.float32)
    v_bias = np.asarray(v_bias, dtype=np.float32)
    rpb_table = np.asarray(rpb_table, dtype=np.float32)
    proj_w = np.asarray(proj_w, dtype=np.float32)
    proj_b = np.asarray(proj_b, dtype=np.float32)
    rel_pos_index = np.asarray(rel_pos_index)

    w_q, w_k, w_v = qkv_w[0:C], qkv_w[C:2 * C], qkv_w[2 * C:3 * C]

    # qk^T weights: q columns pre-scaled; (768, 1536) -> k-tile-major device
    w_qkT = np.concatenate([w_q.T * SCALE, w_k.T], axis=1)
    wqk_dev = _ktile_layout(w_qkT)

    qkb = np.zeros((128, 12), dtype=np.float32)
    for j in range(6):
        qkb[:, j] = q_bias[j * 128:(j + 1) * 128] * SCALE

    # v weights with a zero column after each head's 64 (ones come from vbr)
    w_vT = w_v.T  # (768, 768)
    w_vT_pad = np.zeros((C, VW), dtype=np.float32)
    vbr = np.zeros((1, VW), dtype=np.float32)
    for h in range(H):
        w_vT_pad[:, h * 65:h * 65 + 64] = w_vT[:, h * 64:(h + 1) * 64]
        vbr[0, h * 65:h * 65 + 64] = v_bias[h * 64:(h + 1) * 64]
        vbr[0, h * 65 + 64] = 1.0
    wv_dev = _ktile_layout(w_vT_pad)

    wp_dev = _ktile_layout(np.ascontiguousarray(proj_w.T))
    pbr = proj_b.reshape(1, C)

    # exp(rpb^T): [key, query, head]
    rpb_g = rpb_table[rel_pos_index.reshape(-1)].reshape(N, N, H)
    erT = np.exp(rpb_g.transpose(1, 0, 2))
    rpb0 = np.ascontiguousarray(
        erT[0:128].transpose(0, 2, 1).reshape(128, H * N)).astype(np.float16)
    rpb1 = np.ascontiguousarray(
        erT[128:N].transpose(0, 2, 1).reshape(69, H * N)).astype(np.float16)

    shared = {
        "wqk": wqk_dev, "wv": wv_dev, "wp": wp_dev,
        "rpb0": rpb0, "rpb1": rpb1, "qkb": qkb, "vbr": vbr, "pbr": pbr,
    }

    in_maps = []
    for c in range(NCORES):
        xc = x[c * BC:(c + 1) * BC].reshape(T, C)
        xT = xc.T  # (768, 1576)
        xt_dev = np.ascontiguousarray(
            xT.reshape(KT, 128, PAIRS, TP).transpose(1, 2, 0, 3)
            .reshape(128, PAIRS * KT * TP))
        in_maps.append({"xt": xt_dev, **shared})
    return in_maps


def _ensure_devices():
    import jax

    try:
        if len(jax.devices()) >= NCORES:
            return
    except Exception:
        pass
    try:
        jax.config.update("jax_platforms", "axon")
    except Exception:
        pass


def kernel(x, qkv_w, q_bias, v_bias, rpb_table, proj_w, proj_b,
           rel_pos_index, _trace=False, _trace_kwargs=None):
    _ensure_devices()
    nc = _get_program()
    in_maps = _host_prep(x, qkv_w, q_bias, v_bias, rpb_table, proj_w, proj_b,
                         rel_pos_index)
    res = run_bass_kernel_spmd(
        nc, in_maps, core_ids=list(range(NCORES)),
        trace=_trace, **(_trace_kwargs or {}),
    )
    out = np.concatenate(
        [res.results[c]["out"].reshape(BC, N, C) for c in range(NCORES)], axis=0)
    if _trace:
        kernel._last_results = res
    return out


# revision 31
# speedup vs baseline: 1.0313x; 1.0313x over previous
"""Trainium2 Bass kernel for BEiT-style attention with relative position bias.

Shapes (hardcoded): x (64, 197, 768), 12 heads x 64 dim, rpb table (732, 12).

Sharding: data-parallel over batch -- 8 batches per NeuronCore, weights
replicated. Each core processes its 8 batches in 4 pairs (moving dim 394).

Per-core dataflow (all layouts chosen so no on-device transposes are needed):
  qk^T   = W_qk @ x^T        float32r matmuls, heads pair-packed on partitions
  v_nat  = x @ W_v^T         token-major V with a fused ones-column per head
  s^T    = k_h^T.T @ q_h^T   fp16, keys on partitions
  e      = exp(s^T - 5) * exp(rpb^T)    (softmax max-subtract replaced by a
                                         constant shift; exactly cancels)
  pv     = [v_h | ones].T @ e   -> rows 0:64 unnormalized out^T, row 64 colsum
  out^T  = pv[0:64] * bcast(1/colsum)
  final  = out^T.T @ W_p^T + b  float32r, token-major output
"""

import sys

if "/opt/trn_rl_repo" not in sys.path:
    sys.path.insert(0, "/opt/trn_rl_repo")

import numpy as np

import concourse.bass as bass
import concourse.mybir as mybir
import concourse.tile as tile
from concourse import bacc
from concourse.bass_utils import run_bass_kernel_spmd

F32 = mybir.dt.float32
F16 = mybir.dt.float16
F32R = mybir.dt.float32r
AF = mybir.ActivationFunctionType

B, N, C, H, HD = 64, 197, 768, 12, 64
NCORES = 8
BC = B // NCORES          # batches per core
PAIRS = BC // 2           # batch pairs per core
TP = 2 * N                # tokens per pair (394)
T = BC * N                # tokens per core (1576)
KT = C // 128             # contraction tiles (6)
SCALE = HD ** -0.5
VW = H * (HD + 1)         # v buffer width incl. ones columns (780)
EXP_SHIFT = -5.0


def _r(x):
    return x.bitcast(F32R)


def _ktile_layout(w):
    """(768, M) -> (128, 6*M) with k-tile-major columns."""
    m = w.shape[1]
    return np.ascontiguousarray(
        w.reshape(KT, 128, m).transpose(1, 0, 2).reshape(128, KT * m)
    )


def _build_program():
    nc = bacc.Bacc("TRN2", target_bir_lowering=False, debug=False,
                   num_devices=NCORES)

    xt_d = nc.declare_dram_parameter("xt", [128, PAIRS * KT * TP], F32R, isOutput=False)
    wqk_d = nc.declare_dram_parameter("wqk", [128, KT * 12 * 128], F32R, isOutput=False)
    wv_d = nc.declare_dram_parameter("wv", [128, KT * VW], F32R, isOutput=False)
    wp_d = nc.declare_dram_parameter("wp", [128, KT * C], F32R, isOutput=False)
    rpb0_d = nc.declare_dram_parameter("rpb0", [128, H * N], F16, isOutput=False)
    rpb1_d = nc.declare_dram_parameter("rpb1", [69, H * N], F16, isOutput=False)
    qkb_d = nc.declare_dram_parameter("qkb", [128, 12], F32, isOutput=False)
    vbr_d = nc.declare_dram_parameter("vbr", [1, VW], F32, isOutput=False)
    pbr_d = nc.declare_dram_parameter("pbr", [1, C], F32, isOutput=False)
    out_d = nc.declare_dram_parameter("out", [T, C], F32, isOutput=True)

    from contextlib import ExitStack

    with tile.TileContext(nc) as tc, ExitStack() as ctx:
        consts = ctx.enter_context(tc.tile_pool(name="consts", bufs=1))
        xt_pool = ctx.enter_context(tc.tile_pool(name="xt", bufs=2))
        qk_pool = ctx.enter_context(tc.tile_pool(name="qk", bufs=1))
        v_pool = ctx.enter_context(tc.tile_pool(name="v", bufs=1))
        es_pool = ctx.enter_context(tc.tile_pool(name="es", bufs=1))
        ot_pool = ctx.enter_context(tc.tile_pool(name="ot", bufs=2))
        fs_pool = ctx.enter_context(tc.tile_pool(name="fs", bufs=2))
        rr_pool = ctx.enter_context(tc.tile_pool(name="rr", bufs=2))
        pvs_pool = ctx.enter_context(tc.tile_pool(name="pvs", bufs=8))
        rb_pool = ctx.enter_context(tc.tile_pool(name="rb", bufs=4))
        dram_pool = ctx.enter_context(tc.tile_pool(name="dsc", bufs=4, space="DRAM"))
        ps_mm = ctx.enter_context(tc.tile_pool(name="ps_mm", bufs=3, space="PSUM"))
        ps_sc = ctx.enter_context(tc.tile_pool(name="ps_sc", bufs=3, space="PSUM"))
        ps_pv = ctx.enter_context(tc.tile_pool(name="ps_pv", bufs=2, space="PSUM"))

        if True:
            wqk_t = [consts.tile([128, 12 * 128], F32R, name=f"wqk{k}")
                     for k in range(KT)]
            wv_t = [consts.tile([128, VW], F32R, name=f"wv{k}")
                    for k in range(KT)]
            wp_t = [consts.tile([128, C], F32R, name=f"wp{k}")
                    for k in range(KT)]
            # load order matters: the DMA queue is FIFO, so put the first
            # pair's dependencies (wqk0, then xt in the pair loop) ahead of
            # the bulk weights
            nc.sync.dma_start(wqk_t[0][:], wqk_d[:, 0:1536])
            qkb = consts.tile([128, 12], F32)
            nc.sync.dma_start(qkb[:], qkb_d[:])
            xt0_t = [xt_pool.tile([128, TP], F32R, tag=f"xt{k}",
                                  name=f"xt0{k}") for k in range(KT)]
            for k in range(KT):
                nc.sync.dma_start(xt0_t[k][:], xt_d[:, k * TP:(k + 1) * TP])
            for k in range(1, KT):
                nc.sync.dma_start(wqk_t[k][:],
                                  wqk_d[:, k * 1536:(k + 1) * 1536])
            for k in range(KT):
                nc.sync.dma_start(wv_t[k][:], wv_d[:, k * VW:(k + 1) * VW])
            rpb0 = consts.tile([128, H * N], F16)
            nc.sync.dma_start(rpb0[:], rpb0_d[:])
            rpb1 = consts.tile([69, H * N], F16)
            nc.sync.dma_start(rpb1[:], rpb1_d[:])
            for k in range(KT):
                nc.sync.dma_start(wp_t[k][:], wp_d[:, k * C:(k + 1) * C])
            vbr = consts.tile([128, VW], F32)
            _vb = vbr_d[:]
            nc.sync.dma_start(
                vbr[:],
                bass.AP(tensor=_vb.tensor, offset=_vb.offset,
                        ap=[[0, 128]] + list(_vb.ap[1:])),
            )
            pbr = consts.tile([128, C], F32)
            _pb = pbr_d[:]
            nc.sync.dma_start(
                pbr[:],
                bass.AP(tensor=_pb.tensor, offset=_pb.offset,
                        ap=[[0, 128]] + list(_pb.ap[1:])),
            )
            nb = consts.tile([128, 1], F32)
            nc.vector.memset(nb[:], EXP_SHIFT)

            def _make_proj(p, ot_tiles, half_idx=None):
                tok_tiles = {0: [(0, 128), (128, 69)],
                             1: [(197, 128), (325, 69)]}
                tiles = (tok_tiles[0] + tok_tiles[1]
                         if half_idx is None else tok_tiles[half_idx])

                def emit():
                    for toff, rows in tiles:
                        fs = fs_pool.tile([128, C], F32, tag="fs", name="fs")
                        for half in range(2):
                            pf = ps_mm.tile([128, TP], F32, tag="mm",
                                            name="pf")
                            for k in range(KT):
                                nc.tensor.matmul(
                                    pf[0:rows, 0:384],
                                    ot_tiles[k][:, toff:toff + rows],
                                    wp_t[k][:, half * 384:
                                             (half + 1) * 384],
                                    start=(k == 0), stop=(k == KT - 1),
                                )
                            nc.vector.tensor_add(
                                fs[0:rows, half * 384:(half + 1) * 384],
                                pf[0:rows, 0:384],
                                pbr[0:rows, half * 384:(half + 1) * 384],
                            )
                        nc.sync.dma_start(
                            out_d[p * TP + toff: p * TP + toff + rows, :],
                            fs[0:rows, :],
                        )
                return emit

            pending_proj = None
            for p in range(PAIRS):
                # per-pair x^T load (pair 0 was prefetched before the
                # bulk weight DMAs)
                if p == 0:
                    xt_t = xt0_t
                else:
                    xt_t = [xt_pool.tile([128, TP], F32R, tag=f"xt{k}",
                                         name=f"xt{k}") for k in range(KT)]
                    for k in range(KT):
                        nc.sync.dma_start(
                            xt_t[k][:],
                            xt_d[:, p * KT * TP + k * TP:
                                 p * KT * TP + (k + 1) * TP])

                # ---- qk^T: 12 M-tiles (6 q-pair tiles, then 6 k-pair tiles)
                qk_tiles = []
                for j in range(12):
                    pq = ps_mm.tile([128, TP], F32, tag="mm")
                    for k in range(KT):
                        nc.tensor.matmul(
                            pq[:],
                            wqk_t[k][:, j * 128:(j + 1) * 128],
                            xt_t[k][:],
                            start=(k == 0), stop=(k == KT - 1),
                        )
                    qj = qk_pool.tile([128, TP], F16, tag=f"qk{j}")
                    nc.vector.tensor_scalar_add(qj[:], pq[:], qkb[:, j:j + 1])
                    qk_tiles.append(qj)

                # ---- v natural (+ones cols): 4 token tiles per pair
                v_tiles = {}
                for b2 in range(2):
                    for t, (toff, rows) in enumerate(
                            [(b2 * N, 128), (b2 * N + 128, N - 128)]):
                        vt = v_pool.tile([128, VW], F16, tag=f"v{b2}{t}")
                        for half in range(2):
                            hw = VW // 2
                            pv = ps_mm.tile([128, TP], F32, tag="mm")
                            for k in range(KT):
                                nc.tensor.matmul(
                                    pv[0:rows, 0:hw],
                                    xt_t[k][:, toff:toff + rows],
                                    wv_t[k][:, half * hw:(half + 1) * hw],
                                    start=(k == 0), stop=(k == KT - 1),
                                )
                            nc.vector.tensor_add(
                                vt[0:rows, half * hw:(half + 1) * hw],
                                pv[0:rows, 0:hw],
                                vbr[0:rows, half * hw:(half + 1) * hw],
                            )
                        v_tiles[(b2, t)] = vt

                # ---- attention
                ot_tiles = [ot_pool.tile([128, TP], F32R, tag=f"ot{k}",
                                         name=f"ot{k}")
                            for k in range(KT)]
                for b2 in range(2):
                    boff = b2 * N
                    es_tiles = []
                    for h in range(H):
                        jt, hb = h // 2, (h % 2) * 64
                        psc = ps_sc.tile([128, TP], F32, tag="sc")
                        kt_tile = qk_tiles[6 + jt]
                        q_rhs = qk_tiles[jt][hb:hb + 64, boff:boff + N]
                        nc.tensor.matmul(
                            psc[:, 0:N],
                            kt_tile[hb:hb + 64, boff:boff + 128],
                            q_rhs, start=True, stop=True,
                        )
                        nc.tensor.matmul(
                            psc[0:69, N:2 * N],
                            kt_tile[hb:hb + 64, boff + 128:boff + N],
                            q_rhs, start=True, stop=True,
                        )
                        es = es_pool.tile([128, TP], F16, tag=f"es{h}")
                        nc.scalar.activation(es[:, 0:N], psc[:, 0:N],
                                             AF.Exp, bias=nb[:])
                        nc.scalar.activation(es[0:69, N:2 * N],
                                             psc[0:69, N:2 * N],
                                             AF.Exp, bias=nb[0:69])
                        nc.vector.tensor_mul(
                            es[:, 0:N], es[:, 0:N],
                            rpb0[:, h * N:(h + 1) * N])
                        nc.vector.tensor_mul(
                            es[0:69, N:2 * N], es[0:69, N:2 * N],
                            rpb1[0:69, h * N:(h + 1) * N])
                        es_tiles.append(es)

                    # pass B: 2 heads per PSUM bank; evacuate PSUM -> SBUF
                    # immediately (ACT), normalize later off-critical-path
                    dsc = dram_pool.tile([1, H * N], F32, tag="dsc")
                    pvs_tiles = []
                    for h in range(H):
                        es = es_tiles[h]
                        if h % 2 == 0:
                            ppv = ps_pv.tile([65, TP], F32, tag="pv")
                        coff = (h % 2) * N
                        nc.tensor.matmul(
                            ppv[:, coff:coff + N],
                            v_tiles[(b2, 0)][0:128, h * 65:(h + 1) * 65],
                            es[:, 0:N], start=True, stop=False,
                        )
                        nc.tensor.matmul(
                            ppv[:, coff:coff + N],
                            v_tiles[(b2, 1)][0:69, h * 65:(h + 1) * 65],
                            es[0:69, N:2 * N], start=False, stop=True,
                        )
                        if h % 2 == 1:
                            pvs = pvs_pool.tile([65, TP], F32, tag="pvs")
                            nc.scalar.copy(pvs[:], ppv[:])
                            pvs_tiles.append(pvs)
                            nc.gpsimd.dma_start(
                                dsc[0:1, (h - 1) * N:(h + 1) * N],
                                pvs[64:65, :])
                    # batched reciprocal: DRAM hop to put 12 rows on 12
                    # partitions, then one DVE reciprocal
                    rsb = rr_pool.tile([H, N], F32, tag="rsb")
                    _d = dsc[:]
                    nc.gpsimd.dma_start(
                        rsb[:],
                        bass.AP(tensor=_d.tensor, offset=_d.offset,
                                ap=[[N, H], [1, N]]))
                    rsr = rr_pool.tile([H, N], F32, tag="rsr")
                    nc.vector.reciprocal(rsr[:], rsb[:])
                    dsc2 = dram_pool.tile([H, N], F32, tag="dsc2")
                    nc.gpsimd.dma_start(dsc2[:], rsr[:])
                    for hh in range(H):
                        jt, hb = hh // 2, (hh % 2) * 64
                        rb = rb_pool.tile([64, N], F32, tag="rb")
                        _d2 = dsc2[:]
                        nc.gpsimd.dma_start(
                            rb[:],
                            bass.AP(tensor=_d2.tensor,
                                    offset=_d2.offset + hh * N,
                                    ap=[[0, 64], [1, N]]),
                        )
                        nc.vector.tensor_mul(
                            ot_tiles[jt][hb:hb + 64, boff:boff + N],
                            pvs_tiles[hh // 2][0:64, (hh % 2) * N:
                                               (hh % 2) * N + N],
                            rb[:],
                        )
                    if p == PAIRS - 1 and b2 == 1:
                        # both halves at the very end: proj-A runs on the PE
                        # while b2=1's normalize chain drains, then proj-B
                        _make_proj(p, ot_tiles, half_idx=0)()
                        _make_proj(p, ot_tiles, half_idx=1)()

                if pending_proj is not None:
                    pending_proj()
                    pending_proj = None
                if p < PAIRS - 1:
                    pending_proj = _make_proj(p, ot_tiles)

    nc.compile()
    return nc


_PROGRAM_CACHE = {}


def _get_program():
    if "nc" not in _PROGRAM_CACHE:
        _PROGRAM_CACHE["nc"] = _build_program()
    return _PROGRAM_CACHE["nc"]


def _host_prep(x, qkv_w, q_bias, v_bias, rpb_table, proj_w, proj_b,
               rel_pos_index):
    x = np.asarray(x, dtype=np.float32)
    qkv_w = np.asarray(qkv_w, dtype=np.float32)
    q_bias = np.asarray(q_bias, dtype=np.float32)
    v_bias = np.asarray(v_bias, dtype=np.float32)
    rpb_table = np.asarray(rpb_table, dtype=np.float32)
    proj_w = np.asarray(proj_w, dtype=np.float32)
    proj_b = np.asarray(proj_b, dtype=np.float32)
    rel_pos_index = np.asarray(rel_pos_index)

    w_q, w_k, w_v = qkv_w[0:C], qkv_w[C:2 * C], qkv_w[2 * C:3 * C]

    # qk^T weights: q columns pre-scaled; (768, 1536) -> k-tile-major device
    w_qkT = np.concatenate([w_q.T * SCALE, w_k.T], axis=1)
    wqk_dev = _ktile_layout(w_qkT)

    qkb = np.zeros((128, 12), dtype=np.float32)
    for j in range(6):
        qkb[:, j] = q_bias[j * 128:(j + 1) * 128] * SCALE

    # v weights with a zero column after each head's 64 (ones come from vbr)
    w_vT = w_v.T  # (768, 768)
    w_vT_pad = np.zeros((C, VW), dtype=np.float32)
    vbr = np.zeros((1, VW), dtype=np.float32)
    for h in range(H):
        w_vT_pad[:, h * 65:h * 65 + 64] = w_vT[:, h * 64:(h + 1) * 64]
        vbr[0, h * 65:h * 65 + 64] = v_bias[h * 64:(h + 1) * 64]
        vbr[0, h * 65 + 64] = 1.0
    wv_dev = _ktile_layout(w_vT_pad)

    wp_dev = _ktile_layout(np.ascontiguousarray(proj_w.T))
    pbr = proj_b.reshape(1, C)

    # exp(rpb^T): [key, query, head]
    rpb_g = rpb_table[rel_pos_index.reshape(-1)].reshape(N, N, H)
    erT = np.exp(rpb_g.transpose(1, 0, 2))
    rpb0 = np.ascontiguousarray(
        erT[0:128].transpose(0, 2, 1).reshape(128, H * N)).astype(np.float16)
    rpb1 = np.ascontiguousarray(
        erT[128:N].transpose(0, 2, 1).reshape(69, H * N)).astype(np.float16)

    shared = {
        "wqk": wqk_dev, "wv": wv_dev, "wp": wp_dev,
        "rpb0": rpb0, "rpb1": rpb1, "qkb": qkb, "vbr": vbr, "pbr": pbr,
    }

    in_maps = []
    for c in range(NCORES):
        xc = x[c * BC:(c + 1) * BC].reshape(T, C)
        xT = xc.T  # (768, 1576)
        xt_dev = np.ascontiguousarray(
            xT.reshape(KT, 128, PAIRS, TP).transpose(1, 2, 0, 3)
            .reshape(128, PAIRS * KT * TP))
        in_maps.append({"xt": xt_dev, **shared})
    return in_maps


def _ensure_devices():
    import jax

    try:
        if len(jax.devices()) >= NCORES:
            return
    except Exception:
        pass
    try:
        jax.config.update("jax_platforms", "axon")
    except Exception:
        pass


def kernel(x, qkv_w, q_bias, v_bias, rpb_table, proj_w, proj_b,
           rel_pos_index, _trace=False, _trace_kwargs=None):
    _ensure_devices()
    nc = _get_program()
    in_maps = _host_prep(x, qkv_w, q_bias, v_bias, rpb_table, proj_w, proj_b,
                         rel_pos_index)
    res = run_bass_kernel_spmd(
        nc, in_maps, core_ids=list(range(NCORES)),
        trace=_trace, **(_trace_kwargs or {}),
    )
    out = np.concatenate(
        [res.results[c]["out"].reshape(BC, N, C) for c in range(NCORES)], axis=0)
    if _trace:
        kernel._last_results = res
    return out


# revision 32
# speedup vs baseline: 1.0988x; 1.0655x over previous
"""Trainium2 Bass kernel for BEiT-style attention with relative position bias.

Shapes (hardcoded): x (64, 197, 768), 12 heads x 64 dim, rpb table (732, 12).

Sharding: data-parallel over batch -- 8 batches per NeuronCore, weights
replicated. Each core processes its 8 batches in 4 pairs (moving dim 394).

Per-core dataflow (all layouts chosen so no on-device transposes are needed):
  qk^T   = W_qk @ x^T        float32r matmuls, heads pair-packed on partitions
  v_nat  = x @ W_v^T         token-major V with a fused ones-column per head
  s^T    = k_h^T.T @ q_h^T   fp16, keys on partitions
  e      = exp(s^T - 5) * exp(rpb^T)    (softmax max-subtract replaced by a
                                         constant shift; exactly cancels)
  pv     = [v_h | ones].T @ e   -> rows 0:64 unnormalized out^T, row 64 colsum
  out^T  = pv[0:64] * bcast(1/colsum)
  final  = out^T.T @ W_p^T + b  float32r, token-major output
"""

import sys

if "/opt/trn_rl_repo" not in sys.path:
    sys.path.insert(0, "/opt/trn_rl_repo")

import numpy as np

import concourse.bass as bass
import concourse.mybir as mybir
import concourse.tile as tile
from concourse import bacc
from concourse.bass_utils import run_bass_kernel_spmd

F32 = mybir.dt.float32
F16 = mybir.dt.float16
F32R = mybir.dt.float32r
AF = mybir.ActivationFunctionType

B, N, C, H, HD = 64, 197, 768, 12, 64
NCORES = 8
BC = B // NCORES          # batches per core
PAIRS = BC // 2           # batch pairs per core
TP = 2 * N                # tokens per pair (394)
T = BC * N                # tokens per core (1576)
KT = C // 128             # contraction tiles (6)
SCALE = HD ** -0.5
VW = H * (HD + 1)         # v buffer width incl. ones columns (780)
EXP_SHIFT = -5.0


def _r(x):
    return x.bitcast(F32R)


def _ktile_layout(w):
    """(768, M) -> (128, 6*M) with k-tile-major columns."""
    m = w.shape[1]
    return np.ascontiguousarray(
        w.reshape(KT, 128, m).transpose(1, 0, 2).reshape(128, KT * m)
    )


def _build_program():
    nc = bacc.Bacc("TRN2", target_bir_lowering=False, debug=False,
                   num_devices=NCORES)

    xt_d = nc.declare_dram_parameter("xt", [128, PAIRS * KT * TP], F32R, isOutput=False)
    wqk_d = nc.declare_dram_parameter("wqk", [128, KT * 12 * 128], F32R, isOutput=False)
    wv_d = nc.declare_dram_parameter("wv", [128, KT * VW], F32R, isOutput=False)
    wp_d = nc.declare_dram_parameter("wp", [128, KT * C], F32R, isOutput=False)
    rpb0_d = nc.declare_dram_parameter("rpb0", [128, H * N], F16, isOutput=False)
    rpb1_d = nc.declare_dram_parameter("rpb1", [69, H * N], F16, isOutput=False)
    qkb_d = nc.declare_dram_parameter("qkb", [128, 12], F32, isOutput=False)
    vbr_d = nc.declare_dram_parameter("vbr", [1, VW], F32, isOutput=False)
    pbr_d = nc.declare_dram_parameter("pbr", [1, C], F32, isOutput=False)
    out_d = nc.declare_dram_parameter("out", [T, C], F32, isOutput=True)

    from contextlib import ExitStack

    with tile.TileContext(nc) as tc, ExitStack() as ctx:
        consts = ctx.enter_context(tc.tile_pool(name="consts", bufs=1))
        xt_pool = ctx.enter_context(tc.tile_pool(name="xt", bufs=2))
        qk_pool = ctx.enter_context(tc.tile_pool(name="qk", bufs=1))
        v_pool = ctx.enter_context(tc.tile_pool(name="v", bufs=1))
        es_pool = ctx.enter_context(tc.tile_pool(name="es", bufs=1))
        ot_pool = ctx.enter_context(tc.tile_pool(name="ot", bufs=2))
        fs_pool = ctx.enter_context(tc.tile_pool(name="fs", bufs=2))
        rr_pool = ctx.enter_context(tc.tile_pool(name="rr", bufs=2))
        pvs_pool = ctx.enter_context(tc.tile_pool(name="pvs", bufs=8))
        rb_pool = ctx.enter_context(tc.tile_pool(name="rb", bufs=4))
        dram_pool = ctx.enter_context(tc.tile_pool(name="dsc", bufs=4, space="DRAM"))
        ps_mm = ctx.enter_context(tc.tile_pool(name="ps_mm", bufs=3, space="PSUM"))
        ps_sc = ctx.enter_context(tc.tile_pool(name="ps_sc", bufs=3, space="PSUM"))
        ps_pv = ctx.enter_context(tc.tile_pool(name="ps_pv", bufs=2, space="PSUM"))

        if True:
            wqk_t = [consts.tile([128, 12 * 128], F32R, name=f"wqk{k}")
                     for k in range(KT)]
            wv_t = [consts.tile([128, VW], F32R, name=f"wv{k}")
                    for k in range(KT)]
            wp_t = [consts.tile([128, C], F32R, name=f"wp{k}")
                    for k in range(KT)]
            # load order matters: the DMA queue is FIFO, so put the first
            # pair's dependencies (wqk0, then xt in the pair loop) ahead of
            # the bulk weights
            nc.sync.dma_start(wqk_t[0][:], wqk_d[:, 0:1536])
            qkb = consts.tile([128, 12], F32)
            nc.sync.dma_start(qkb[:], qkb_d[:])
            xt0_t = [xt_pool.tile([128, TP], F32R, tag=f"xt{k}",
                                  name=f"xt0{k}") for k in range(KT)]
            for k in range(KT):
                nc.sync.dma_start(xt0_t[k][:], xt_d[:, k * TP:(k + 1) * TP])
            for k in range(1, KT):
                nc.sync.dma_start(wqk_t[k][:],
                                  wqk_d[:, k * 1536:(k + 1) * 1536])
            for k in range(KT):
                nc.sync.dma_start(wv_t[k][:], wv_d[:, k * VW:(k + 1) * VW])
            rpb0 = consts.tile([128, H * N], F16)
            nc.sync.dma_start(rpb0[:], rpb0_d[:])
            rpb1 = consts.tile([69, H * N], F16)
            nc.sync.dma_start(rpb1[:], rpb1_d[:])
            for k in range(KT):
                nc.sync.dma_start(wp_t[k][:], wp_d[:, k * C:(k + 1) * C])
            vbr = consts.tile([128, VW], F32)
            _vb = vbr_d[:]
            nc.sync.dma_start(
                vbr[:],
                bass.AP(tensor=_vb.tensor, offset=_vb.offset,
                        ap=[[0, 128]] + list(_vb.ap[1:])),
            )
            pbr = consts.tile([128, C], F32)
            _pb = pbr_d[:]
            nc.sync.dma_start(
                pbr[:],
                bass.AP(tensor=_pb.tensor, offset=_pb.offset,
                        ap=[[0, 128]] + list(_pb.ap[1:])),
            )
            nb = consts.tile([128, 1], F32)
            nc.vector.memset(nb[:], EXP_SHIFT)

            def _make_proj(p, ot_tiles, half_idx=None):
                tok_tiles = {0: [(0, 128), (128, 69)],
                             1: [(197, 128), (325, 69)]}
                tiles = (tok_tiles[0] + tok_tiles[1]
                         if half_idx is None else tok_tiles[half_idx])

                def emit():
                    for toff, rows in tiles:
                        fs = fs_pool.tile([128, C], F32, tag="fs", name="fs")
                        for half in range(2):
                            pf = ps_mm.tile([128, TP], F32, tag="mm",
                                            name="pf")
                            for k in range(KT):
                                nc.tensor.matmul(
                                    pf[0:rows, 0:384],
                                    ot_tiles[k][:, toff:toff + rows],
                                    wp_t[k][:, half * 384:
                                             (half + 1) * 384],
                                    start=(k == 0), stop=(k == KT - 1),
                                )
                            nc.vector.tensor_add(
                                fs[0:rows, half * 384:(half + 1) * 384],
                                pf[0:rows, 0:384],
                                pbr[0:rows, half * 384:(half + 1) * 384],
                            )
                        nc.sync.dma_start(
                            out_d[p * TP + toff: p * TP + toff + rows, :],
                            fs[0:rows, :],
                        )
                return emit

            pending_proj = None
            for p in range(PAIRS):
                # per-pair x^T load (pair 0 was prefetched before the
                # bulk weight DMAs)
                if p == 0:
                    xt_t = xt0_t
                else:
                    xt_t = [xt_pool.tile([128, TP], F32R, tag=f"xt{k}",
                                         name=f"xt{k}") for k in range(KT)]
                    for k in range(KT):
                        nc.sync.dma_start(
                            xt_t[k][:],
                            xt_d[:, p * KT * TP + k * TP:
                                 p * KT * TP + (k + 1) * TP])

                # ---- qk^T: 12 M-tiles (6 q-pair tiles, then 6 k-pair tiles)
                qk_tiles = []
                for j in range(12):
                    pq = ps_mm.tile([128, TP], F32, tag="mm")
                    for k in range(KT):
                        nc.tensor.matmul(
                            pq[:],
                            wqk_t[k][:, j * 128:(j + 1) * 128],
                            xt_t[k][:],
                            start=(k == 0), stop=(k == KT - 1),
                        )
                    qj = qk_pool.tile([128, TP], F16, tag=f"qk{j}")
                    nc.vector.tensor_scalar_add(qj[:], pq[:], qkb[:, j:j + 1])
                    qk_tiles.append(qj)

                # ---- v natural (+ones cols): 4 token tiles per pair
                v_tiles = {}
                for b2 in range(2):
                    for t, (toff, rows) in enumerate(
                            [(b2 * N, 128), (b2 * N + 128, N - 128)]):
                        vt = v_pool.tile([128, VW], F16, tag=f"v{b2}{t}")
                        for half in range(2):
                            hw = VW // 2
                            pv = ps_mm.tile([128, TP], F32, tag="mm")
                            for k in range(KT):
                                nc.tensor.matmul(
                                    pv[0:rows, 0:hw],
                                    xt_t[k][:, toff:toff + rows],
                                    wv_t[k][:, half * hw:(half + 1) * hw],
                                    start=(k == 0), stop=(k == KT - 1),
                                )
                            nc.vector.tensor_add(
                                vt[0:rows, half * hw:(half + 1) * hw],
                                pv[0:rows, 0:hw],
                                vbr[0:rows, half * hw:(half + 1) * hw],
                            )
                        v_tiles[(b2, t)] = vt

                # ---- attention
                ot_tiles = [ot_pool.tile([128, TP], F32R, tag=f"ot{k}",
                                         name=f"ot{k}")
                            for k in range(KT)]
                for b2 in range(2):
                    boff = b2 * N
                    es_tiles = []
                    for h in range(H):
                        jt, hb = h // 2, (h % 2) * 64
                        psc = ps_sc.tile([128, TP], F32, tag="sc")
                        kt_tile = qk_tiles[6 + jt]
                        q_rhs = qk_tiles[jt][hb:hb + 64, boff:boff + N]
                        nc.tensor.matmul(
                            psc[:, 0:N],
                            kt_tile[hb:hb + 64, boff:boff + 128],
                            q_rhs, start=True, stop=True,
                        )
                        nc.tensor.matmul(
                            psc[0:69, N:2 * N],
                            kt_tile[hb:hb + 64, boff + 128:boff + N],
                            q_rhs, start=True, stop=True,
                        )
                        es = es_pool.tile([128, TP], F16, tag=f"es{h}")
                        nc.scalar.activation(es[:, 0:N], psc[:, 0:N],
                                             AF.Exp, bias=nb[:])
                        nc.scalar.activation(es[0:69, N:2 * N],
                                             psc[0:69, N:2 * N],
                                             AF.Exp, bias=nb[0:69])
                        nc.vector.tensor_mul(
                            es[:, 0:N], es[:, 0:N],
                            rpb0[:, h * N:(h + 1) * N])
                        nc.vector.tensor_mul(
                            es[0:69, N:2 * N], es[0:69, N:2 * N],
                            rpb1[0:69, h * N:(h + 1) * N])
                        es_tiles.append(es)

                    # pass B: 2 heads per PSUM bank; evacuate PSUM -> SBUF
                    # immediately (ACT), normalize later off-critical-path
                    dsc = dram_pool.tile([1, H * N], F32, tag="dsc")
                    pvs_tiles = []
                    for h in range(H):
                        es = es_tiles[h]
                        if h % 2 == 0:
                            ppv = ps_pv.tile([65, TP], F32, tag="pv")
                        coff = (h % 2) * N
                        nc.tensor.matmul(
                            ppv[:, coff:coff + N],
                            v_tiles[(b2, 0)][0:128, h * 65:(h + 1) * 65],
                            es[:, 0:N], start=True, stop=False,
                        )
                        nc.tensor.matmul(
                            ppv[:, coff:coff + N],
                            v_tiles[(b2, 1)][0:69, h * 65:(h + 1) * 65],
                            es[0:69, N:2 * N], start=False, stop=True,
                        )
                        if h % 2 == 1:
                            pvs = pvs_pool.tile([65, TP], F32, tag="pvs")
                            nc.scalar.copy(pvs[:], ppv[:])
                            pvs_tiles.append(pvs)
                            nc.gpsimd.dma_start(
                                dsc[0:1, (h - 1) * N:(h + 1) * N],
                                pvs[64:65, :])
                    # batched reciprocal: DRAM hop to put 12 rows on 12
                    # partitions, then one DVE reciprocal
                    rsb = rr_pool.tile([H, N], F32, tag="rsb")
                    _d = dsc[:]
                    nc.gpsimd.dma_start(
                        rsb[:],
                        bass.AP(tensor=_d.tensor, offset=_d.offset,
                                ap=[[N, H], [1, N]]))
                    rsr = rr_pool.tile([H, N], F32, tag="rsr")
                    nc.vector.reciprocal(rsr[:], rsb[:])
                    dsc2 = dram_pool.tile([H, N], F32, tag="dsc2")
                    nc.gpsimd.dma_start(dsc2[:], rsr[:])
                    for hh in range(H):
                        jt, hb = hh // 2, (hh % 2) * 64
                        rb = rb_pool.tile([64, N], F32, tag="rb")
                        _d2 = dsc2[:]
                        nc.gpsimd.dma_start(
                            rb[:],
                            bass.AP(tensor=_d2.tensor,
                                    offset=_d2.offset + hh * N,
                                    ap=[[0, 64], [1, N]]),
                        )
                        nc.vector.tensor_mul(
                            ot_tiles[jt][hb:hb + 64, boff:boff + N],
                            pvs_tiles[hh // 2][0:64, (hh % 2) * N:
                                               (hh % 2) * N + N],
                            rb[:],
                        )
                    if p == PAIRS - 1 and b2 == 1:
                        # flush the deferred previous-pair proj first, then
                        # this pair's halves: proj work covers the PE while
                        # the final normalize chain drains
                        if pending_proj is not None:
                            pending_proj()
                            pending_proj = None
                        _make_proj(p, ot_tiles, half_idx=0)()
                        _make_proj(p, ot_tiles, half_idx=1)()

                if pending_proj is not None:
                    pending_proj()
                    pending_proj = None
                if p < PAIRS - 1:
                    pending_proj = _make_proj(p, ot_tiles)

    nc.compile()
    return nc


_PROGRAM_CACHE = {}


def _get_program():
    if "nc" not in _PROGRAM_CACHE:
        _PROGRAM_CACHE["nc"] = _build_program()
    return _PROGRAM_CACHE["nc"]


def _host_prep(x, qkv_w, q_bias, v_bias, rpb_table, proj_w, proj_b,
               rel_pos_index):
    x = np.asarray(x, dtype=np.float32)
    qkv_w = np.asarray(qkv_w, dtype=np.float32)
    q_bias = np.asarray(q_bias, dtype=np.float32)
    v_bias = np.asarray(v_bias, dtype=np.float32)
    rpb_table = np.asarray(rpb_table, dtype=np.float32)
    proj_w = np.asarray(proj_w, dtype=np.float32)
    proj_b = np.asarray(proj_b, dtype=np.float32)
    rel_pos_index = np.asarray(rel_pos_index)

    w_q, w_k, w_v = qkv_w[0:C], qkv_w[C:2 * C], qkv_w[2 * C:3 * C]

    # qk^T weights: q columns pre-scaled; (768, 1536) -> k-tile-major device
    w_qkT = np.concatenate([w_q.T * SCALE, w_k.T], axis=1)
    wqk_dev = _ktile_layout(w_qkT)

    qkb = np.zeros((128, 12), dtype=np.float32)
    for j in range(6):
        qkb[:, j] = q_bias[j * 128:(j + 1) * 128] * SCALE

    # v weights with a zero column after each head's 64 (ones come from vbr)
    w_vT = w_v.T  # (768, 768)
    w_vT_pad = np.zeros((C, VW), dtype=np.float32)
    vbr = np.zeros((1, VW), dtype=np.float32)
    for h in range(H):
        w_vT_pad[:, h * 65:h * 65 + 64] = w_vT[:, h * 64:(h + 1) * 64]
        vbr[0, h * 65:h * 65 + 64] = v_bias[h * 64:(h + 1) * 64]
        vbr[0, h * 65 + 64] = 1.0
    wv_dev = _ktile_layout(w_vT_pad)

    wp_dev = _ktile_layout(np.ascontiguousarray(proj_w.T))
    pbr = proj_b.reshape(1, C)

    # exp(rpb^T): [key, query, head]
    rpb_g = rpb_table[rel_pos_index.reshape(-1)].reshape(N, N, H)
    erT = np.exp(rpb_g.transpose(1, 0, 2))
    rpb0 = np.ascontiguousarray(
        erT[0:128].transpose(0, 2, 1).reshape(128, H * N)).astype(np.float16)
    rpb1 = np.ascontiguousarray(
        erT[128:N].transpose(0, 2, 1).reshape(69, H * N)).astype(np.float16)

    shared = {
        "wqk": wqk_dev, "wv": wv_dev, "wp": wp_dev,
        "rpb0": rpb0, "rpb1": rpb1, "qkb": qkb, "vbr": vbr, "pbr": pbr,
    }

    in_maps = []
    for c in range(NCORES):
        xc = x[c * BC:(c + 1) * BC].reshape(T, C)
        xT = xc.T  # (768, 1576)
        xt_dev = np.ascontiguousarray(
            xT.reshape(KT, 128, PAIRS, TP).transpose(1, 2, 0, 3)
            .reshape(128, PAIRS * KT * TP))
        in_maps.append({"xt": xt_dev, **shared})
    return in_maps


def _ensure_devices():
    import jax

    try:
        if len(jax.devices()) >= NCORES:
            return
    except Exception:
        pass
    try:
        jax.config.update("jax_platforms", "axon")
    except Exception:
        pass


def kernel(x, qkv_w, q_bias, v_bias, rpb_table, proj_w, proj_b,
           rel_pos_index, _trace=False, _trace_kwargs=None):
    _ensure_devices()
    nc = _get_program()
    in_maps = _host_prep(x, qkv_w, q_bias, v_bias, rpb_table, proj_w, proj_b,
                         rel_pos_index)
    res = run_bass_kernel_spmd(
        nc, in_maps, core_ids=list(range(NCORES)),
        trace=_trace, **(_trace_kwargs or {}),
    )
    out = np.concatenate(
        [res.results[c]["out"].reshape(BC, N, C) for c in range(NCORES)], axis=0)
    if _trace:
        kernel._last_results = res
    return out
